# revision 11
# baseline (speedup 1.0000x reference)
"""Trainium2 Bass kernel for nn_ChunkStrategy (chunk-size selection head).

Strategy: pure data parallel over the batch dim (16 batches -> 8 cores x 2).
All heavy traffic is the edge_features stream ([B,S,S,De] f32, 604 MB total);
each core streams its 2 batches in [128, S, De] row tiles. Per tile:
  - ScalarE activation(Copy) with accum_out   -> per-row sum over (j,k)  (edge_summary)
  - VectorE tensor_tensor_reduce(add -> min)  -> masked min distance over j (k=0 plane)
Everything downstream is tiny per-batch math done on-chip; output is one f32
chunk size per batch.

The small linear layers are folded on the host (pure parameter preprocessing):
  importance = sigmoid(nf . v_node + sum_k ve_k * relu(es * w1_k + b1_k) + c0) * mask
with v_node = W_node @ W_imp[:32], ve = W_e2 @ W_imp[32:],
     c0 = b_node @ W_imp[:32] + b_e2 @ W_imp[32:] + b_imp.
"""

import numpy as np

# Problem shapes (hardcoded per the task contract).
B, S, H, De = 16, 768, 128, 16
N_CORES = 8
BPC = B // N_CORES            # batches per core
P = 128                       # SBUF partitions
T = S // P                    # 6 i-tiles per batch
JK = S * De                   # 12288 elements summed per row
BIG = 1.0e9                   # penalty for non-pocket columns
DIST_CORE = 6.0
DIST_SHELL = 10.0
ADJ = 64.0                    # min(BASE_CHUNK, MAX_SEQ_LEN//4) for S=768
MAX_SEQ_LEN = 256.0

F32MAX = 3.0e38


def _build_nc(c0: float, reps: int = 1):
    import contextlib

    import concourse.bacc as bacc
    import concourse.bass as bass
    import concourse.tile as tile
    from concourse import mybir

    f32 = mybir.dt.float32
    Alu = mybir.AluOpType
    Act = mybir.ActivationFunctionType

    nc = bacc.Bacc("TRN2", target_bir_lowering=False)

    edge = nc.dram_tensor("edge", [BPC, S, S, De], f32, kind="ExternalInput")
    nf = nc.dram_tensor("nf", [BPC, S, H], f32, kind="ExternalInput")
    pocket = nc.dram_tensor("pocket", [BPC, S], f32, kind="ExternalInput")
    smask = nc.dram_tensor("smask", [BPC, S], f32, kind="ExternalInput")
    vnode = nc.dram_tensor("vnode", [H], f32, kind="ExternalInput")
    w1s = nc.dram_tensor("w1s", [64], f32, kind="ExternalInput")  # W_e1[0]/JK
    b1 = nc.dram_tensor("b1", [64], f32, kind="ExternalInput")
    ve = nc.dram_tensor("ve", [64], f32, kind="ExternalInput")
    out = nc.dram_tensor("chunk_out", [BPC, 1], f32, kind="ExternalOutput")

    def bcast(ap, p=P):
        # zero-stride partition broadcast of a DRAM row
        return bass.AP(tensor=ap.tensor, offset=ap.offset, ap=[[0, p]] + list(ap.ap))

    with tile.TileContext(nc) as tc:
        with (
            tc.tile_pool(name="consts", bufs=1) as consts,
            tc.tile_pool(name="p_edge", bufs=2) as p_edge,
            tc.tile_pool(name="p_scr", bufs=2) as p_scr,
            tc.tile_pool(name="p_nf", bufs=2) as p_nf,
            tc.tile_pool(name="p_mlp", bufs=2) as p_mlp,
            tc.tile_pool(name="p_pen", bufs=2) as p_pen,
            tc.tile_pool(name="p_pack", bufs=2) as p_pack,
            tc.tile_pool(name="p_small", bufs=2) as p_small,
            tc.tile_pool(name="p_psum", bufs=2, space="PSUM") as p_psum,
        ):
            # --- constants, loaded once ---
            vnodeb = consts.tile([P, H], f32)
            nc.gpsimd.dma_start(out=vnodeb, in_=bcast(vnode[:]))
            w1sb = consts.tile([P, 64], f32)
            nc.gpsimd.dma_start(out=w1sb, in_=bcast(w1s[:]))
            b1b = consts.tile([P, 64], f32)
            nc.gpsimd.dma_start(out=b1b, in_=bcast(b1[:]))
            veb = consts.tile([P, 64], f32)
            nc.gpsimd.dma_start(out=veb, in_=bcast(ve[:]))
            ones128 = consts.tile([P, 1], f32)
            nc.vector.memset(ones128, 1.0)
            c0b = consts.tile([P, 1], f32)
            nc.vector.memset(c0b, c0)

            # reps>1 wraps the whole body in a hardware loop — used only by
            # the timing harness to measure steady-state per-iteration time.
            loop_ctx = (
                tc.For_i(0, reps, 1) if reps > 1 else contextlib.nullcontext()
            )
            with loop_ctx:
                _emit_body(nc, tc, locals())

    nc.finalize()
    return nc


def _emit_body(nc, tc, env):
    import concourse.bass as bass
    from concourse import mybir

    f32 = mybir.dt.float32
    Alu = mybir.AluOpType
    Act = mybir.ActivationFunctionType

    edge, nf, pocket, smask, out = (
        env["edge"], env["nf"], env["pocket"], env["smask"], env["out"]
    )
    vnodeb, w1sb, b1b, veb, ones128, c0b = (
        env["vnodeb"], env["w1sb"], env["b1b"], env["veb"],
        env["ones128"], env["c0b"],
    )
    p_edge, p_scr, p_nf, p_mlp, p_pen, p_pack, p_small, p_psum = (
        env["p_edge"], env["p_scr"], env["p_nf"], env["p_mlp"],
        env["p_pen"], env["p_pack"], env["p_small"], env["p_psum"],
    )
    bcast = env["bcast"]

    if True:
            for b in range(BPC):
                # --- per-batch small loads ---
                pocketb = p_pen.tile([P, S], f32)
                nc.gpsimd.dma_start(out=pocketb, in_=bcast(pocket[b]))
                penb = p_pen.tile([P, S], f32)
                # (1 - pocket) * BIG  ==  pocket * (-BIG) + BIG
                nc.vector.tensor_scalar(
                    out=penb, in0=pocketb, scalar1=-BIG, scalar2=BIG,
                    op0=Alu.mult, op1=Alu.add,
                )
                pocket_all = p_pack.tile([P, T], f32)
                nc.gpsimd.dma_start(
                    out=pocket_all, in_=pocket[b].rearrange("(t p) -> p t", p=P)
                )
                smask_all = p_pack.tile([P, T], f32)
                nc.gpsimd.dma_start(
                    out=smask_all, in_=smask[b].rearrange("(t p) -> p t", p=P)
                )

                es_all = p_pack.tile([P, T], f32)      # raw sums over (j,k)
                dist_all = p_pack.tile([P, T], f32)    # masked min distance
                ncon_all = p_pack.tile([P, T], f32)    # nf . v_node
                edot_all = p_pack.tile([P, T], f32)    # sum_k ve_k relu(...)

                for t in range(T):
                    sl = slice(t * P, (t + 1) * P)
                    etile = p_edge.tile([P, S, De], f32)
                    nc.sync.dma_start(out=etile, in_=edge[b, sl])

                    # edge row sum over (j,k): in-place Copy + accumulate
                    nc.scalar.activation(
                        out=etile, in_=etile, func=Act.Copy,
                        accum_out=es_all[:, t : t + 1],
                    )

                    # masked min over j of edge[...,0] + penalty
                    minscr = p_scr.tile([P, S], f32)
                    nc.vector.tensor_add(minscr, etile[:, :, 0], penb)
                    nc.vector.tensor_reduce(
                        out=dist_all[:, t : t + 1], in_=minscr,
                        axis=mybir.AxisListType.X, op=Alu.min,
                    )

                    # node contribution: sum_h nf[i,h] * v_node[h]
                    nftile = p_nf.tile([P, H], f32)
                    nc.sync.dma_start(out=nftile, in_=nf[b, sl])
                    nfscr = p_nf.tile([P, H], f32)
                    nc.vector.tensor_mul(nfscr, nftile, vnodeb)
                    nc.vector.tensor_reduce(
                        out=ncon_all[:, t : t + 1], in_=nfscr,
                        axis=mybir.AxisListType.X, op=Alu.add,
                    )

                    # tiny MLP on es: edot = sum_k ve_k * relu(es*w1s_k + b1_k)
                    h1 = p_mlp.tile([P, 64], f32)
                    nc.vector.tensor_single_scalar(
                        out=h1, in_=w1sb, scalar=es_all[:, t : t + 1], op=Alu.mult
                    )
                    h2 = p_mlp.tile([P, 64], f32)
                    nc.vector.tensor_add(h2, h1, b1b)
                    h3 = p_mlp.tile([P, 64], f32)
                    nc.scalar.activation(out=h3, in_=h2, func=Act.Relu)
                    h4 = p_mlp.tile([P, 64], f32)
                    nc.vector.tensor_mul(h4, h3, veb)
                    nc.vector.tensor_reduce(
                        out=edot_all[:, t : t + 1], in_=h4,
                        axis=mybir.AxisListType.X, op=Alu.add,
                    )

                # --- per-batch epilogue ---
                imp_pre = p_pack.tile([P, T], f32)
                nc.vector.tensor_add(imp_pre, ncon_all, edot_all)
                imp_sig = p_pack.tile([P, T], f32)
                nc.scalar.activation(
                    out=imp_sig, in_=imp_pre, func=Act.Sigmoid, bias=c0b[:, 0:1]
                )
                imp_all = p_pack.tile([P, T], f32)
                nc.vector.tensor_mul(imp_all, imp_sig, smask_all)

                # cross-partition sums via ones-matmul -> [1, T] psum rows
                pi = p_psum.tile([1, T], f32)
                nc.tensor.matmul(pi, ones128, imp_all)
                imp_sum = p_small.tile([1, 1], f32)
                nc.vector.tensor_reduce(
                    out=imp_sum, in_=pi, axis=mybir.AxisListType.X, op=Alu.add
                )
                imp_mean = p_small.tile([1, 1], f32)
                nc.vector.tensor_scalar_mul(imp_mean, imp_sum, 1.0 / S)
                meanb = p_small.tile([P, 1], f32)
                nc.gpsimd.partition_broadcast(meanb, imp_mean)

                core_all = p_pack.tile([P, T], f32)
                nc.vector.tensor_single_scalar(
                    out=core_all, in_=dist_all, scalar=DIST_CORE, op=Alu.is_lt
                )
                band1 = p_pack.tile([P, T], f32)
                nc.vector.tensor_single_scalar(
                    out=band1, in_=dist_all, scalar=DIST_CORE, op=Alu.is_ge
                )
                band2 = p_pack.tile([P, T], f32)
                nc.vector.tensor_single_scalar(
                    out=band2, in_=dist_all, scalar=DIST_SHELL, op=Alu.is_lt
                )
                sgt = p_pack.tile([P, T], f32)
                nc.vector.tensor_single_scalar(
                    out=sgt, in_=imp_all, scalar=meanb, op=Alu.is_gt
                )
                shell1 = p_pack.tile([P, T], f32)
                nc.vector.tensor_mul(shell1, band1, band2)
                shell2 = p_pack.tile([P, T], f32)
                nc.vector.tensor_mul(shell2, shell1, sgt)
                merged1 = p_pack.tile([P, T], f32)
                nc.vector.tensor_max(merged1, pocket_all, core_all)
                merged2 = p_pack.tile([P, T], f32)
                nc.vector.tensor_max(merged2, merged1, shell2)

                pm = p_psum.tile([1, T], f32)
                nc.tensor.matmul(pm, ones128, merged2)
                msum = p_small.tile([1, 1], f32)
                nc.vector.tensor_reduce(
                    out=msum, in_=pm, axis=mybir.AxisListType.X, op=Alu.add
                )
                pp = p_psum.tile([1, T], f32)
                nc.tensor.matmul(pp, ones128, pocket_all)
                pcnt = p_small.tile([1, 1], f32)
                nc.vector.tensor_reduce(
                    out=pcnt, in_=pp, axis=mybir.AxisListType.X, op=Alu.add
                )

                # chunk_pocket = min(max(merged_sum, ADJ), MAX_SEQ_LEN)
                cp = p_small.tile([1, 1], f32)
                nc.vector.tensor_scalar(
                    out=cp, in0=msum, scalar1=ADJ, scalar2=MAX_SEQ_LEN,
                    op0=Alu.max, op1=Alu.min,
                )
                # chunk_np = clip(64 * imp_mean, 32, 128)
                cnp1 = p_small.tile([1, 1], f32)
                nc.vector.tensor_scalar(
                    out=cnp1, in0=imp_mean, scalar1=64.0, scalar2=ADJ / 2,
                    op0=Alu.mult, op1=Alu.max,
                )
                cnp = p_small.tile([1, 1], f32)
                nc.vector.tensor_single_scalar(
                    out=cnp, in_=cnp1, scalar=min(ADJ * 2, MAX_SEQ_LEN), op=Alu.min
                )
                hp = p_small.tile([1, 1], f32)
                nc.vector.tensor_single_scalar(
                    out=hp, in_=pcnt, scalar=0.5, op=Alu.is_gt
                )
                # chunk = cnp + hp * (cp - cnp)
                dlt = p_small.tile([1, 1], f32)
                nc.vector.tensor_sub(dlt, cp, cnp)
                pr = p_small.tile([1, 1], f32)
                nc.vector.tensor_mul(pr, hp, dlt)
                chunk = p_small.tile([1, 1], f32)
                nc.vector.tensor_add(chunk, cnp, pr)

                nc.sync.dma_start(out=out[b : b + 1, :], in_=chunk)


def _fold_params(W_node, b_node, W_e1, b_e1, W_e2, b_e2, W_imp, b_imp):
    W_node = np.asarray(W_node, np.float32)
    b_node = np.asarray(b_node, np.float32)
    W_e1 = np.asarray(W_e1, np.float32)
    b_e1 = np.asarray(b_e1, np.float32)
    W_e2 = np.asarray(W_e2, np.float32)
    b_e2 = np.asarray(b_e2, np.float32)
    W_imp = np.asarray(W_imp, np.float32)
    b_imp = np.asarray(b_imp, np.float32)
    wa = W_imp[:32, 0]
    wb = W_imp[32:, 0]
    vnode = (W_node @ wa).astype(np.float32)                  # [128]
    ve = (W_e2 @ wb).astype(np.float32)                       # [64]
    c0 = float(b_node @ wa + b_e2 @ wb + b_imp[0])
    w1s = (W_e1[0, :] / np.float32(JK)).astype(np.float32)    # [64]
    return vnode, ve, c0, w1s, b_e1


def run(inputs, trace=False, trace_cores=None):
    from concourse.bass_utils import run_bass_kernel_spmd

    node_features = np.ascontiguousarray(inputs["node_features"], np.float32)
    edge_features = np.ascontiguousarray(inputs["edge_features"], np.float32)
    sequence_mask = np.ascontiguousarray(inputs["sequence_mask"], np.float32)
    pocket_mask = np.ascontiguousarray(inputs["pocket_mask"]).astype(np.float32)

    vnode, ve, c0, w1s, b1 = _fold_params(
        inputs["W_node"], inputs["b_node"], inputs["W_e1"], inputs["b_e1"],
        inputs["W_e2"], inputs["b_e2"], inputs["W_imp"], inputs["b_imp"],
    )

    nc = _build_nc(c0)

    in_maps = []
    for c in range(N_CORES):
        bs = slice(c * BPC, (c + 1) * BPC)
        in_maps.append(
            dict(
                edge=edge_features[bs],
                nf=node_features[bs],
                pocket=pocket_mask[bs],
                smask=sequence_mask[bs],
                vnode=vnode,
                w1s=w1s,
                b1=b1,
                ve=ve,
            )
        )

    kw = {}
    if trace:
        kw["trace"] = True
        kw["trace_cores"] = trace_cores if trace_cores is not None else [0]
    res = run_bass_kernel_spmd(nc, in_maps, core_ids=list(range(N_CORES)), **kw)
    chunks = np.concatenate(
        [r["chunk_out"].reshape(-1) for r in res.results]
    ).astype(np.float32)
    return chunks, res


def kernel(**inputs):
    chunks, _ = run(inputs, trace=False)
    return chunks, 256


# ---------------------------------------------------------------------------
# Timing harness (used by test.py only): execute with device-resident inputs
# so repeated calls measure dispatch + on-device time without host transfer.
# ---------------------------------------------------------------------------

def timed_exec(nc, in_maps, n_calls=8):
    import time

    import jax
    import numpy as np
    from jax.experimental.shard_map import shard_map
    from jax.sharding import Mesh, NamedSharding, PartitionSpec

    from concourse import mybir
    from concourse.bass2jax import _bass_exec_p, install_neuronx_cc_hook

    from concourse.bass2jax import partition_id_tensor

    install_neuronx_cc_hook()
    n_cores = len(in_maps)

    partition_name = nc.partition_id_tensor.name if nc.partition_id_tensor else None
    in_names, out_names, out_avals, zero_outs = [], [], [], []
    for alloc in nc.m.functions[0].allocations:
        if not isinstance(alloc, mybir.MemoryLocationSet):
            continue
        name = alloc.memorylocations[0].name
        if alloc.kind == "ExternalInput":
            if name != partition_name:
                in_names.append(name)
        elif alloc.kind == "ExternalOutput":
            shape = tuple(alloc.tensor_shape)
            dtype = mybir.dt.np(alloc.dtype)
            out_names.append(name)
            out_avals.append(jax.core.ShapedArray(shape, dtype))
            zero_outs.append(np.zeros(shape, dtype))
    n_params = len(in_names)
    n_outs = len(out_names)
    all_names = in_names + out_names
    if partition_name is not None:
        all_names = all_names + [partition_name]
    donate = tuple(range(n_params, n_params + n_outs))

    def _body(*args):
        operands = list(args)
        if partition_name is not None:
            operands.append(partition_id_tensor())
        outs = _bass_exec_p.bind(
            *operands,
            out_avals=tuple(out_avals),
            in_names=tuple(all_names),
            out_names=tuple(out_names),
            lowering_input_output_aliases=(),
            sim_require_finite=True,
            sim_require_nnan=True,
            nc=nc,
        )
        return tuple(outs)

    devices = jax.devices()[:n_cores]
    mesh = Mesh(np.asarray(devices), ("core",))
    spec = PartitionSpec("core")
    sharded = jax.jit(
        shard_map(
            _body, mesh=mesh,
            in_specs=(spec,) * (n_params + n_outs),
            out_specs=(spec,) * n_outs,
            check_rep=False,
        ),
        donate_argnums=donate, keep_unused=True,
    )
    concat_in = [
        np.concatenate([in_maps[c][name] for c in range(n_cores)], axis=0)
        for name in in_names
    ]
    sh = NamedSharding(mesh, spec)
    din = [jax.device_put(a, sh) for a in concat_in]

    def one_call():
        zs = [
            jax.device_put(np.zeros((n_cores * z.shape[0], *z.shape[1:]), z.dtype), sh)
            for z in zero_outs
        ]
        t0 = time.perf_counter()
        outs = sharded(*din, *zs)
        jax.block_until_ready(outs)
        return time.perf_counter() - t0, outs

    one_call()  # warmup + compile
    times = []
    outs = None
    for _ in range(n_calls):
        dt, outs = one_call()
        times.append(dt)
    per_core = [
        {
            name: np.asarray(outs[i]).reshape(n_cores, *out_avals[i].shape)[c]
            for i, name in enumerate(out_names)
        }
        for c in range(n_cores)
    ]
    return times, per_core
